# revision 2
# baseline (speedup 1.0000x reference)
"""Multi-head attention kernel for Trainium2 (Bass/Tile), 8 NeuronCores.

Problem (per batch b of 8, one batch per core):
    Q = q @ wq_w.T + wq_b ; K = k @ wk_w.T + wk_b ; V = v @ wv_w.T + wv_b
    per head h (12 heads, depth 64):
        S_h = Q_h @ K_h.T / sqrt(768)
        P_h = softmax(S_h)            -> output attn_w [12,1024,1024]
        O_h = P_h @ V_h
    attention = concat(O) @ dense_w.T + dense_b   -> output [1024,768]

Sharding: data-parallel over batch, core i <- batch i. No collectives.

Per-core plan (all matmuls fp32r = full-rate fp32-reduced on the PE):
  - PE-transpose q,k,v and the 4 weight matrices into emb-major layout.
  - Form-1 projections give Q^T,K^T [768,1024] (emb-major: head h lives in
    chunk h//2, partition half (h%2)*64); Form-2 gives V token-major,
    stored ones-augmented as V_aug [lk, 12*65] (65th col of each head = 1).
  - Pass 1 scores S = Q_h K_h^T in [lq,lk] layout; ACT evicts with fused
    Exp(scale*x) + accum_out row sums; DVE normalizes -> attn_w.
  - Pass 2 scores S^T = K_h Q_h^T in [lk,lq] layout; ACT evicts with Exp
    -> E^T (unnormalized, fp32r).
  - O_aug^T[65, lq] = V_aug_h^T E^T accumulated over lk chunks in PSUM;
    row 64 = softmax sums; reciprocal + gpsimd partition_broadcast + DVE
    multiply evicts normalized O^T into concat^T.
  - Dense: Form-2 matmul over concat^T chunks, bias via a K=1 matmul with
    a ones row; evict + DMA out.
"""

import math

import numpy as np

import concourse.bacc as bacc
import concourse.tile as tile
from concourse import mybir
from concourse.bass_utils import run_bass_kernel_spmd
from concourse.masks import make_identity

P = 128          # partitions
L = 1024         # sequence length
E = 768          # embedding dim
H = 12           # heads
D = 64           # head depth
EC = E // P      # 6 emb chunks
LC = L // P      # 8 seq chunks
NB = 2           # 384-wide output chunks for V/dense projections
F32 = mybir.dt.float32
F32R = mybir.dt.float32r
AF = mybir.ActivationFunctionType
OP = mybir.AluOpType
SCALE = 1.0 / math.sqrt(E)
N_CORES = 8


def _build():
    nc = bacc.Bacc("TRN2", target_bir_lowering=False)
    dq = nc.dram_tensor("q", [L, E], F32, kind="ExternalInput")
    dk = nc.dram_tensor("k", [L, E], F32, kind="ExternalInput")
    dv = nc.dram_tensor("v", [L, E], F32, kind="ExternalInput")
    dwq = nc.dram_tensor("wq_w", [E, E], F32, kind="ExternalInput")
    dwk = nc.dram_tensor("wk_w", [E, E], F32, kind="ExternalInput")
    dwv = nc.dram_tensor("wv_w", [E, E], F32, kind="ExternalInput")
    dwd = nc.dram_tensor("dense_w", [E, E], F32, kind="ExternalInput")
    dbq = nc.dram_tensor("wq_b", [E], F32, kind="ExternalInput")
    dbk = nc.dram_tensor("wk_b", [E], F32, kind="ExternalInput")
    dbv = nc.dram_tensor("wv_b", [1, E], F32, kind="ExternalInput")
    dbd = nc.dram_tensor("dense_b", [1, E], F32, kind="ExternalInput")
    datt = nc.dram_tensor("attention", [L, E], F32, kind="ExternalOutput")
    dw = nc.dram_tensor("attn_w", [H, L, L], F32, kind="ExternalOutput")

    with tile.TileContext(nc) as tc:
        with (
            tc.tile_pool(name="consts", bufs=1) as consts,
            tc.tile_pool(name="persist", bufs=1) as persist,
            tc.tile_pool(name="psA", bufs=2, space="PSUM") as psA,   # [128,1024] scores pass 1
            tc.tile_pool(name="psB", bufs=2, space="PSUM") as psB,   # [128,512] transposes/proj/pass2
            tc.tile_pool(name="psC", bufs=2, space="PSUM") as psC,   # [128,512] V/dense/O
        ):
            ident = consts.tile([P, P], F32)
            make_identity(nc, ident)
            ones_f = consts.tile([1, P], F32)
            nc.vector.memset(ones_f[:], 1.0)
            ones_row = consts.tile([1, P], F32R)
            nc.vector.tensor_copy(out=ones_row[:], in_=ones_f[:])
            bq_t = consts.tile([P, EC], F32)
            nc.sync.dma_start(out=bq_t[:], in_=dbq[:].rearrange("(c p) -> p c", p=P))
            bk_t = consts.tile([P, EC], F32)
            nc.sync.dma_start(out=bk_t[:], in_=dbk[:].rearrange("(c p) -> p c", p=P))
            bv_row = consts.tile([1, E], F32R)
            nc.gpsimd.dma_start(out=bv_row[:], in_=dbv[:])
            bd_row = consts.tile([1, E], F32R)
            nc.gpsimd.dma_start(out=bd_row[:], in_=dbd[:])

            QT = persist.tile([P, EC, L], F32R, tag="QT", name="QT")
            KT = persist.tile([P, EC, L], F32R, tag="KT", name="KT")
            Vg = persist.tile([P, LC, H * 65], F32R, tag="Vg", name="Vg")
            CT = persist.tile([P, EC, L], F32R, tag="CT", name="CT")
            DT = persist.tile([P, EC, E], F32R, tag="DT", name="DT")
            Vg4 = Vg[:].rearrange("p c (h x) -> p c h x", x=65)
            vones_f = consts.tile([P, LC, H, 1], F32)
            nc.vector.memset(vones_f[:], 1.0)
            nc.vector.tensor_copy(out=Vg4[:, :, :, 64:65], in_=vones_f[:])

            def transpose_into(dst, src, name):
                """dst[:128,:128] = src.T via PE; evict (casts to dst dtype)."""
                pt = psB.tile([P, 512], F32, tag="s2", name=f"pt_{name}")
                nc.tensor.transpose(pt[:, 0:P], src, ident[:])
                nc.any.tensor_copy(out=dst, in_=pt[:, 0:P])

            # ---------------- phase A/B: transposes + projections ----------------
            with (
                tc.tile_pool(name="nat", bufs=3) as natp,
                tc.tile_pool(name="xt", bufs=1) as xtp,
                tc.tile_pool(name="wt", bufs=2) as wtp,
            ):
                def load_wT(dram_w, dst, name):
                    """dst[128, EC, E] (fp32r) = dram_w.T ([in, out] emb-major)."""
                    for c in range(EC):
                        natw = natp.tile([P, E], F32, tag="nat", name=f"{name}_nat{c}")
                        nc.sync.dma_start(out=natw[:], in_=dram_w[c * P:(c + 1) * P, :])
                        for j in range(EC):
                            transpose_into(dst[:, j, c * P:(c + 1) * P],
                                           natw[:, j * P:(j + 1) * P], f"{name}{c}{j}")

                def load_xT(dram_x, dst, name):
                    """dst[128, EC, L] (fp32r) = dram_x.T (emb-major activations)."""
                    for lc in range(LC):
                        natx = natp.tile([P, E], F32, tag="nat", name=f"{name}_nat{lc}")
                        nc.sync.dma_start(out=natx[:], in_=dram_x[lc * P:(lc + 1) * P, :])
                        for j in range(EC):
                            transpose_into(dst[:, j, lc * P:(lc + 1) * P],
                                           natx[:, j * P:(j + 1) * P], f"{name}{lc}{j}")

                load_wT(dwd, DT, "wd")

                # Q
                wq_t = wtp.tile([P, EC, E], F32R, tag="wt", name="wq_t")
                load_wT(dwq, wq_t, "wq")
                q_t = xtp.tile([P, EC, L], F32R, tag="xt", name="q_t")
                load_xT(dq, q_t, "q")
                for oc in range(EC):
                    for nl in range(2):
                        ps = psB.tile([P, 512], F32, tag="s2", name="qproj")
                        for ic in range(EC):
                            nc.tensor.matmul(ps[:], wq_t[:, ic, oc * P:(oc + 1) * P],
                                             q_t[:, ic, nl * 512:(nl + 1) * 512],
                                             start=(ic == 0), stop=(ic == EC - 1))
                        nc.scalar.activation(out=QT[:, oc, nl * 512:(nl + 1) * 512],
                                             in_=ps[:], func=AF.Identity,
                                             bias=bq_t[:, oc:oc + 1], scale=1.0)

                # K
                wk_t = wtp.tile([P, EC, E], F32R, tag="wt", name="wk_t")
                load_wT(dwk, wk_t, "wk")
                k_t = xtp.tile([P, EC, L], F32R, tag="xt", name="k_t")
                load_xT(dk, k_t, "k")
                for oc in range(EC):
                    for nl in range(2):
                        ps = psB.tile([P, 512], F32, tag="s2", name="kproj")
                        for ic in range(EC):
                            nc.tensor.matmul(ps[:], wk_t[:, ic, oc * P:(oc + 1) * P],
                                             k_t[:, ic, nl * 512:(nl + 1) * 512],
                                             start=(ic == 0), stop=(ic == EC - 1))
                        nc.scalar.activation(out=KT[:, oc, nl * 512:(nl + 1) * 512],
                                             in_=ps[:], func=AF.Identity,
                                             bias=bk_t[:, oc:oc + 1], scale=1.0)

                # V (token-major, ones-augmented). Bias enters as a K=1 matmul.
                wv_t = wtp.tile([P, EC, E], F32R, tag="wt", name="wv_t")
                load_wT(dwv, wv_t, "wv")
                v_t = xtp.tile([P, EC, L], F32R, tag="xt", name="v_t")
                load_xT(dv, v_t, "v")
                for lc in range(LC):
                    for nb in range(NB):
                        ps = psC.tile([P, 512], F32, tag="o", name="vproj")
                        for ic in range(EC):
                            nc.tensor.matmul(ps[:, 0:384], v_t[:, ic, lc * P:(lc + 1) * P],
                                             wv_t[:, ic, nb * 384:(nb + 1) * 384],
                                             start=(ic == 0), stop=False)
                        nc.tensor.matmul(ps[:, 0:384], ones_row[:],
                                         bv_row[:, nb * 384:(nb + 1) * 384],
                                         start=False, stop=True)
                        nc.vector.tensor_copy(
                            out=Vg4[:, lc, nb * 6:(nb + 1) * 6, 0:64],
                            in_=ps[:, 0:384].rearrange("p (h d) -> p h d", d=D))

            # ---------------- phase C: attention, per head pair ----------------
            with (
                tc.tile_pool(name="et", bufs=2) as etp,
                tc.tile_pool(name="ep", bufs=2) as epool,
                tc.tile_pool(name="pn", bufs=3) as pnp,
                tc.tile_pool(name="sm", bufs=6) as smp,
                tc.tile_pool(name="bc", bufs=2) as bcp,
                tc.tile_pool(name="att", bufs=2) as attp,
            ):
                for hp in range(H // 2):
                    # heads 2hp (partitions 0:64) and 2hp+1 (64:128), chunk hp
                    for lc in range(LC):
                        psa = psA.tile([P, L], F32, tag="s1", name="s1a")
                        psb_ = psA.tile([P, L], F32, tag="s1", name="s1b")
                        for half in (0, 512):
                            nc.tensor.matmul(psa[:, half:half + 512],
                                             QT[0:64, hp, lc * P:(lc + 1) * P],
                                             KT[0:64, hp, half:half + 512],
                                             start=True, stop=True)
                            nc.tensor.matmul(psb_[:, half:half + 512],
                                             QT[64:128, hp, lc * P:(lc + 1) * P],
                                             KT[64:128, hp, half:half + 512],
                                             start=True, stop=True)
                        for hi, ps_ in ((0, psa), (1, psb_)):
                            h = 2 * hp + hi
                            Et = epool.tile([P, L], F32, tag="E", name="E")
                            sums = smp.tile([P, 1], F32, tag="sums", name="sums")
                            nc.scalar.activation(out=Et[:], in_=ps_[:], func=AF.Exp,
                                                 scale=SCALE, accum_out=sums[:])
                            recip = smp.tile([P, 1], F32, tag="recip", name="recip")
                            nc.vector.reciprocal(recip[:], sums[:])
                            Pn = pnp.tile([P, L], F32, tag="Pn", name="Pn")
                            nc.vector.tensor_scalar_mul(Pn[:], Et[:], recip[:])
                            nc.sync.dma_start(out=dw[h, lc * P:(lc + 1) * P, :], in_=Pn[:])

                    for nl in range(2):
                        ETa = etp.tile([P, LC, 512], F32R, tag="ET", name="ETa")
                        ETb = etp.tile([P, LC, 512], F32R, tag="ET", name="ETb")
                        for kc in range(LC):
                            p2a = psB.tile([P, 512], F32, tag="s2", name="s2a")
                            p2b = psB.tile([P, 512], F32, tag="s2", name="s2b")
                            nc.tensor.matmul(p2a[:], KT[0:64, hp, kc * P:(kc + 1) * P],
                                             QT[0:64, hp, nl * 512:(nl + 1) * 512],
                                             start=True, stop=True)
                            nc.tensor.matmul(p2b[:], KT[64:128, hp, kc * P:(kc + 1) * P],
                                             QT[64:128, hp, nl * 512:(nl + 1) * 512],
                                             start=True, stop=True)
                            nc.scalar.activation(out=ETa[:, kc, :], in_=p2a[:],
                                                 func=AF.Exp, scale=SCALE)
                            nc.scalar.activation(out=ETb[:, kc, :], in_=p2b[:],
                                                 func=AF.Exp, scale=SCALE)
                        for hi, ET in ((0, ETa), (1, ETb)):
                            h = 2 * hp + hi
                            pso = psC.tile([P, 512], F32, tag="o", name="pso")
                            for kc in range(LC):
                                nc.tensor.matmul(pso[0:65, :], Vg[:, kc, h * 65:(h + 1) * 65],
                                                 ET[:, kc, :],
                                                 start=(kc == 0), stop=(kc == LC - 1))
                            rrow = smp.tile([1, 512], F32, tag="rrow", name="rrow")
                            nc.vector.reciprocal(rrow[:], pso[64:65, :])
                            bca = bcp.tile([D, 512], F32, tag="bc", name="bc")
                            nc.gpsimd.partition_broadcast(bca[:], rrow[:])
                            nc.vector.tensor_tensor(
                                CT[hi * D:(hi + 1) * D, hp, nl * 512:(nl + 1) * 512],
                                pso[0:D, :], bca[:], OP.mult)

                # ---------------- phase D: dense projection ----------------
                for lc in range(LC):
                    att_sb = attp.tile([P, E], F32, tag="att", name="att")
                    for nb in range(NB):
                        psd = psC.tile([P, 512], F32, tag="o", name="psd")
                        for ic in range(EC):
                            nc.tensor.matmul(psd[:, 0:384], CT[:, ic, lc * P:(lc + 1) * P],
                                             DT[:, ic, nb * 384:(nb + 1) * 384],
                                             start=(ic == 0), stop=False)
                        nc.tensor.matmul(psd[:, 0:384], ones_row[:],
                                         bd_row[:, nb * 384:(nb + 1) * 384],
                                         start=False, stop=True)
                        nc.vector.tensor_copy(out=att_sb[:, nb * 384:(nb + 1) * 384],
                                              in_=psd[:, 0:384])
                    nc.sync.dma_start(out=datt[lc * P:(lc + 1) * P, :], in_=att_sb[:])

    nc.compile()
    return nc


_NC = None


def _get_nc():
    global _NC
    if _NC is None:
        _NC = _build()
    return _NC


def make_in_maps(**inputs):
    q = np.ascontiguousarray(inputs["q"], dtype=np.float32)
    k = np.ascontiguousarray(inputs["k"], dtype=np.float32)
    v = np.ascontiguousarray(inputs["v"], dtype=np.float32)
    shared = {
        "wq_w": np.ascontiguousarray(inputs["wq_w"], dtype=np.float32),
        "wk_w": np.ascontiguousarray(inputs["wk_w"], dtype=np.float32),
        "wv_w": np.ascontiguousarray(inputs["wv_w"], dtype=np.float32),
        "dense_w": np.ascontiguousarray(inputs["dense_w"], dtype=np.float32),
        "wq_b": np.ascontiguousarray(inputs["wq_b"], dtype=np.float32),
        "wk_b": np.ascontiguousarray(inputs["wk_b"], dtype=np.float32),
        "wv_b": np.ascontiguousarray(inputs["wv_b"], dtype=np.float32).reshape(1, E),
        "dense_b": np.ascontiguousarray(inputs["dense_b"], dtype=np.float32).reshape(1, E),
    }
    return [
        {"q": np.ascontiguousarray(q[i]), "k": np.ascontiguousarray(k[i]),
         "v": np.ascontiguousarray(v[i]), **shared}
        for i in range(N_CORES)
    ]


def kernel(**inputs):
    nc = _get_nc()
    in_maps = make_in_maps(**inputs)
    res = run_bass_kernel_spmd(nc, in_maps, core_ids=list(range(N_CORES)))
    attention = np.stack([res.results[i]["attention"] for i in range(N_CORES)])
    attn_w = np.stack([res.results[i]["attn_w"] for i in range(N_CORES)])
    return attention, attn_w


# revision 3
# speedup vs baseline: 1.0709x; 1.0709x over previous
"""Multi-head attention kernel for Trainium2 (Bass/Tile), 8 NeuronCores.

Problem (per batch b of 8, one batch per core):
    Q = q @ wq_w.T + wq_b ; K = k @ wk_w.T + wk_b ; V = v @ wv_w.T + wv_b
    per head h (12 heads, depth 64):
        S_h = Q_h @ K_h.T / sqrt(768)
        P_h = softmax(S_h)            -> output attn_w [12,1024,1024]
        O_h = P_h @ V_h
    attention = concat(O) @ dense_w.T + dense_b   -> output [1024,768]

Sharding: data-parallel over batch, core i <- batch i. No collectives.

Per-core plan (all matmuls fp32r = full-rate fp32-reduced on the PE):
  - PE-transpose q,k,v and the 4 weight matrices into emb-major layout.
  - Form-1 projections give Q^T,K^T [768,1024] (emb-major: head h lives in
    chunk h//2, partition half (h%2)*64); Form-2 gives V token-major,
    stored ones-augmented as V_aug [lk, 12*65] (65th col of each head = 1).
  - Pass 1 scores S = Q_h K_h^T in [lq,lk] layout; ACT evicts with fused
    Exp(scale*x) + accum_out row sums; DVE normalizes -> attn_w.
  - Pass 2 scores S^T = K_h Q_h^T in [lk,lq] layout; ACT evicts with Exp
    -> E^T (unnormalized, fp32r).
  - O_aug^T[65, lq] = V_aug_h^T E^T accumulated over lk chunks in PSUM;
    row 64 = softmax sums; reciprocal + gpsimd partition_broadcast + DVE
    multiply evicts normalized O^T into concat^T.
  - Dense: Form-2 matmul over concat^T chunks, bias via a K=1 matmul with
    a ones row; evict + DMA out.
"""

import math

import numpy as np

import concourse.bacc as bacc
import concourse.tile as tile
from concourse import mybir
from concourse.bass_utils import run_bass_kernel_spmd
from concourse.masks import make_identity

P = 128          # partitions
L = 1024         # sequence length
E = 768          # embedding dim
H = 12           # heads
D = 64           # head depth
EC = E // P      # 6 emb chunks
LC = L // P      # 8 seq chunks
NB = 2           # 384-wide output chunks for V/dense projections
F32 = mybir.dt.float32
F32R = mybir.dt.float32r
AF = mybir.ActivationFunctionType
OP = mybir.AluOpType
SCALE = 1.0 / math.sqrt(E)
N_CORES = 8


def _build():
    nc = bacc.Bacc("TRN2", target_bir_lowering=False)
    dq = nc.dram_tensor("q", [L, E], F32, kind="ExternalInput")
    dk = nc.dram_tensor("k", [L, E], F32, kind="ExternalInput")
    dv = nc.dram_tensor("v", [L, E], F32, kind="ExternalInput")
    dwq = nc.dram_tensor("wq_w", [E, E], F32, kind="ExternalInput")
    dwk = nc.dram_tensor("wk_w", [E, E], F32, kind="ExternalInput")
    dwv = nc.dram_tensor("wv_w", [E, E], F32, kind="ExternalInput")
    dwd = nc.dram_tensor("dense_w", [E, E], F32, kind="ExternalInput")
    dbq = nc.dram_tensor("wq_b", [E], F32, kind="ExternalInput")
    dbk = nc.dram_tensor("wk_b", [E], F32, kind="ExternalInput")
    dbv = nc.dram_tensor("wv_b", [1, E], F32, kind="ExternalInput")
    dbd = nc.dram_tensor("dense_b", [1, E], F32, kind="ExternalInput")
    datt = nc.dram_tensor("attention", [L, E], F32, kind="ExternalOutput")
    dw = nc.dram_tensor("attn_w", [H, L, L], F32, kind="ExternalOutput")

    with tile.TileContext(nc) as tc:
        with (
            tc.tile_pool(name="consts", bufs=1) as consts,
            tc.tile_pool(name="persist", bufs=1) as persist,
            tc.tile_pool(name="psW", bufs=3, space="PSUM") as psW,   # [128,1024] scores p1+p2
            tc.tile_pool(name="psS", bufs=2, space="PSUM") as psS,   # [128,512] everything else
        ):
            ident = consts.tile([P, P], F32)
            make_identity(nc, ident)
            ones_f = consts.tile([1, P], F32)
            nc.vector.memset(ones_f[:], 1.0)
            ones_row = consts.tile([1, P], F32R)
            nc.vector.tensor_copy(out=ones_row[:], in_=ones_f[:])
            bq_t = consts.tile([P, EC], F32)
            nc.sync.dma_start(out=bq_t[:], in_=dbq[:].rearrange("(c p) -> p c", p=P))
            bk_t = consts.tile([P, EC], F32)
            nc.sync.dma_start(out=bk_t[:], in_=dbk[:].rearrange("(c p) -> p c", p=P))
            bv_row = consts.tile([1, E], F32R)
            nc.gpsimd.dma_start(out=bv_row[:], in_=dbv[:])
            bd_row = consts.tile([1, E], F32R)
            nc.gpsimd.dma_start(out=bd_row[:], in_=dbd[:])

            QT = persist.tile([P, EC, L], F32R, tag="QT", name="QT")
            KT = persist.tile([P, EC, L], F32R, tag="KT", name="KT")
            Vg = persist.tile([P, LC, H * 65], F32R, tag="Vg", name="Vg")
            CT = persist.tile([P, EC, L], F32R, tag="CT", name="CT")
            DT = persist.tile([P, EC, E], F32R, tag="DT", name="DT")
            Vg4 = Vg[:].rearrange("p c (h x) -> p c h x", x=65)
            vones_f = consts.tile([P, LC, H, 1], F32)
            nc.vector.memset(vones_f[:], 1.0)
            nc.vector.tensor_copy(out=Vg4[:, :, :, 64:65], in_=vones_f[:])

            def transpose_into(dst, src, name):
                """dst[:128,:128] = src.T via PE; evict on DVE (casts to dst dtype)."""
                pt = psS.tile([P, 512], F32, tag="s", name=f"pt_{name}")
                nc.tensor.transpose(pt[:, 0:P], src, ident[:])
                nc.vector.tensor_copy(out=dst, in_=pt[:, 0:P])

            # ---------------- phase A/B: transposes + projections ----------------
            with (
                tc.tile_pool(name="nat", bufs=3) as natp,
                tc.tile_pool(name="xt", bufs=1) as xtp,
                tc.tile_pool(name="wt", bufs=2) as wtp,
            ):
                def load_wT(dram_w, dst, name):
                    """dst[128, EC, E] (fp32r) = dram_w.T ([in, out] emb-major)."""
                    for c in range(EC):
                        natw = natp.tile([P, E], F32, tag="nat", name=f"{name}_nat{c}")
                        nc.sync.dma_start(out=natw[:], in_=dram_w[c * P:(c + 1) * P, :])
                        for j in range(EC):
                            transpose_into(dst[:, j, c * P:(c + 1) * P],
                                           natw[:, j * P:(j + 1) * P], f"{name}{c}{j}")

                def load_xT(dram_x, dst, name):
                    """dst[128, EC, L] (fp32r) = dram_x.T (emb-major activations)."""
                    for lc in range(LC):
                        natx = natp.tile([P, E], F32, tag="nat", name=f"{name}_nat{lc}")
                        nc.sync.dma_start(out=natx[:], in_=dram_x[lc * P:(lc + 1) * P, :])
                        for j in range(EC):
                            transpose_into(dst[:, j, lc * P:(lc + 1) * P],
                                           natx[:, j * P:(j + 1) * P], f"{name}{lc}{j}")

                load_wT(dwd, DT, "wd")

                # Q
                wq_t = wtp.tile([P, EC, E], F32R, tag="wt", name="wq_t")
                load_wT(dwq, wq_t, "wq")
                q_t = xtp.tile([P, EC, L], F32R, tag="xt", name="q_t")
                load_xT(dq, q_t, "q")
                for oc in range(EC):
                    for nl in range(2):
                        ps = psS.tile([P, 512], F32, tag="s", name="qproj")
                        for ic in range(EC):
                            nc.tensor.matmul(ps[:], wq_t[:, ic, oc * P:(oc + 1) * P],
                                             q_t[:, ic, nl * 512:(nl + 1) * 512],
                                             start=(ic == 0), stop=(ic == EC - 1))
                        nc.vector.tensor_scalar_add(QT[:, oc, nl * 512:(nl + 1) * 512],
                                                    ps[:], bq_t[:, oc:oc + 1])

                # K
                wk_t = wtp.tile([P, EC, E], F32R, tag="wt", name="wk_t")
                load_wT(dwk, wk_t, "wk")
                k_t = xtp.tile([P, EC, L], F32R, tag="xt", name="k_t")
                load_xT(dk, k_t, "k")
                for oc in range(EC):
                    for nl in range(2):
                        ps = psS.tile([P, 512], F32, tag="s", name="kproj")
                        for ic in range(EC):
                            nc.tensor.matmul(ps[:], wk_t[:, ic, oc * P:(oc + 1) * P],
                                             k_t[:, ic, nl * 512:(nl + 1) * 512],
                                             start=(ic == 0), stop=(ic == EC - 1))
                        nc.vector.tensor_scalar_add(KT[:, oc, nl * 512:(nl + 1) * 512],
                                                    ps[:], bk_t[:, oc:oc + 1])

                # V (token-major, ones-augmented). Bias enters as a K=1 matmul.
                wv_t = wtp.tile([P, EC, E], F32R, tag="wt", name="wv_t")
                load_wT(dwv, wv_t, "wv")
                v_t = xtp.tile([P, EC, L], F32R, tag="xt", name="v_t")
                load_xT(dv, v_t, "v")
                for lc in range(LC):
                    for nb in range(NB):
                        ps = psS.tile([P, 512], F32, tag="s", name="vproj")
                        for ic in range(EC):
                            nc.tensor.matmul(ps[:, 0:384], v_t[:, ic, lc * P:(lc + 1) * P],
                                             wv_t[:, ic, nb * 384:(nb + 1) * 384],
                                             start=(ic == 0), stop=False)
                        nc.tensor.matmul(ps[:, 0:384], ones_row[:],
                                         bv_row[:, nb * 384:(nb + 1) * 384],
                                         start=False, stop=True)
                        nc.vector.tensor_copy(
                            out=Vg4[:, lc, nb * 6:(nb + 1) * 6, 0:64],
                            in_=ps[:, 0:384].rearrange("p (h d) -> p h d", d=D))

            # ---------------- phase C: attention, per head pair ----------------
            with (
                tc.tile_pool(name="et", bufs=1) as etp,
                tc.tile_pool(name="ep", bufs=2) as epool,
                tc.tile_pool(name="pn", bufs=3) as pnp,
                tc.tile_pool(name="sm", bufs=6) as smp,
                tc.tile_pool(name="bc", bufs=2) as bcp,
                tc.tile_pool(name="att", bufs=2) as attp,
            ):
                for hp in range(H // 2):
                    # heads 2hp (partitions 0:64) and 2hp+1 (64:128), chunk hp
                    for lc in range(LC):
                        psa = psW.tile([P, L], F32, tag="w", name="s1a")
                        psb_ = psW.tile([P, L], F32, tag="w", name="s1b")
                        for half in (0, 512):
                            nc.tensor.matmul(psa[:, half:half + 512],
                                             QT[0:64, hp, lc * P:(lc + 1) * P],
                                             KT[0:64, hp, half:half + 512],
                                             start=True, stop=True)
                            nc.tensor.matmul(psb_[:, half:half + 512],
                                             QT[64:128, hp, lc * P:(lc + 1) * P],
                                             KT[64:128, hp, half:half + 512],
                                             start=True, stop=True)
                        for hi, ps_ in ((0, psa), (1, psb_)):
                            h = 2 * hp + hi
                            Et = epool.tile([P, L], F32, tag="E", name="E")
                            sums = smp.tile([P, 1], F32, tag="sums", name="sums")
                            nc.scalar.activation(out=Et[:], in_=ps_[:], func=AF.Exp,
                                                 scale=SCALE, accum_out=sums[:])
                            recip = smp.tile([P, 1], F32, tag="recip", name="recip")
                            nc.vector.reciprocal(recip[:], sums[:])
                            Pn = pnp.tile([P, L], F32, tag="Pn", name="Pn")
                            nc.vector.tensor_scalar_mul(Pn[:], Et[:], recip[:])
                            nc.sync.dma_start(out=dw[h, lc * P:(lc + 1) * P, :], in_=Pn[:])

                    for nl in range(2):
                        ET2 = etp.tile([P, LC, L], F32R, tag="ET", name="ET2")
                        for kc in range(LC):
                            p2 = psW.tile([P, L], F32, tag="w", name="s2m")
                            nc.tensor.matmul(p2[:, 0:512], KT[0:64, hp, kc * P:(kc + 1) * P],
                                             QT[0:64, hp, nl * 512:(nl + 1) * 512],
                                             start=True, stop=True)
                            nc.tensor.matmul(p2[:, 512:1024], KT[64:128, hp, kc * P:(kc + 1) * P],
                                             QT[64:128, hp, nl * 512:(nl + 1) * 512],
                                             start=True, stop=True)
                            nc.scalar.activation(out=ET2[:, kc, :], in_=p2[:],
                                                 func=AF.Exp, scale=SCALE)
                        for hi in (0, 1):
                            h = 2 * hp + hi
                            pso = psS.tile([P, 512], F32, tag="s", name="pso")
                            for kc in range(LC):
                                nc.tensor.matmul(pso[0:65, :], Vg[:, kc, h * 65:(h + 1) * 65],
                                                 ET2[:, kc, hi * 512:(hi + 1) * 512],
                                                 start=(kc == 0), stop=(kc == LC - 1))
                            rrow = smp.tile([1, 512], F32, tag="rrow", name="rrow")
                            nc.vector.reciprocal(rrow[:], pso[64:65, :])
                            bca = bcp.tile([D, 512], F32, tag="bc", name="bc")
                            nc.gpsimd.partition_broadcast(bca[:], rrow[:])
                            nc.vector.tensor_tensor(
                                CT[hi * D:(hi + 1) * D, hp, nl * 512:(nl + 1) * 512],
                                pso[0:D, :], bca[:], OP.mult)

                # ---------------- phase D: dense projection ----------------
                for lc in range(LC):
                    att_sb = attp.tile([P, E], F32, tag="att", name="att")
                    for nb in range(NB):
                        psd = psS.tile([P, 512], F32, tag="s", name="psd")
                        for ic in range(EC):
                            nc.tensor.matmul(psd[:, 0:384], CT[:, ic, lc * P:(lc + 1) * P],
                                             DT[:, ic, nb * 384:(nb + 1) * 384],
                                             start=(ic == 0), stop=False)
                        nc.tensor.matmul(psd[:, 0:384], ones_row[:],
                                         bd_row[:, nb * 384:(nb + 1) * 384],
                                         start=False, stop=True)
                        nc.vector.tensor_copy(out=att_sb[:, nb * 384:(nb + 1) * 384],
                                              in_=psd[:, 0:384])
                    nc.sync.dma_start(out=datt[lc * P:(lc + 1) * P, :], in_=att_sb[:])

    nc.compile()
    return nc


_NC = None


def _get_nc():
    global _NC
    if _NC is None:
        _NC = _build()
    return _NC


def make_in_maps(**inputs):
    q = np.ascontiguousarray(inputs["q"], dtype=np.float32)
    k = np.ascontiguousarray(inputs["k"], dtype=np.float32)
    v = np.ascontiguousarray(inputs["v"], dtype=np.float32)
    shared = {
        "wq_w": np.ascontiguousarray(inputs["wq_w"], dtype=np.float32),
        "wk_w": np.ascontiguousarray(inputs["wk_w"], dtype=np.float32),
        "wv_w": np.ascontiguousarray(inputs["wv_w"], dtype=np.float32),
        "dense_w": np.ascontiguousarray(inputs["dense_w"], dtype=np.float32),
        "wq_b": np.ascontiguousarray(inputs["wq_b"], dtype=np.float32),
        "wk_b": np.ascontiguousarray(inputs["wk_b"], dtype=np.float32),
        "wv_b": np.ascontiguousarray(inputs["wv_b"], dtype=np.float32).reshape(1, E),
        "dense_b": np.ascontiguousarray(inputs["dense_b"], dtype=np.float32).reshape(1, E),
    }
    return [
        {"q": np.ascontiguousarray(q[i]), "k": np.ascontiguousarray(k[i]),
         "v": np.ascontiguousarray(v[i]), **shared}
        for i in range(N_CORES)
    ]


def kernel(**inputs):
    nc = _get_nc()
    in_maps = make_in_maps(**inputs)
    res = run_bass_kernel_spmd(nc, in_maps, core_ids=list(range(N_CORES)))
    attention = np.stack([res.results[i]["attention"] for i in range(N_CORES)])
    attn_w = np.stack([res.results[i]["attn_w"] for i in range(N_CORES)])
    return attention, attn_w
